# revision 1
# baseline (speedup 1.0000x reference)
"""Deformable feature aggregation kernel for 8 Trainium2 NeuronCores.

Strategy (self-contained, hardcoded for the fixed problem shapes):
  - Shard (bs=2 x NA=900) -> 8 cores x 225 anchors; core c handles batch
    c//4, anchor block c%4. Feature maps: each core gets its batch's half.
  - Host prep (layout only): feature maps are repacked into a zero-padded
    "quad" table: one row per bilinear fetch start (y0,x0) holding the 4
    corner pixels' 256-ch vectors contiguously (4KB). One gather descriptor
    fetches a quarter-row (one corner, 1KB).
  - Device: keypoint/projection/softmax matmuls on PE; per-level pixel math
    on DVE; indirect-DMA gather with bounds-check skip for out-of-image
    points; fusion reduce on PE (fusion weights as stationary [sq,8],
    gathered features streamed [sq,256]); masked extract + ones-matmul
    builds fusedT for the output projection; residual add; DMA out.
"""

import numpy as np
from contextlib import ExitStack

import concourse.bass as bass
import concourse.mybir as mybir
import concourse.tile as tile
from concourse.tile import TileContext
from concourse.masks import make_identity
from concourse.bass_utils import run_bass_kernel_spmd

F32 = mybir.dt.float32
I32 = mybir.dt.int32
OP = mybir.AluOpType
ACTF = mybir.ActivationFunctionType

# problem constants
C, L, P, G = 6, 4, 13, 8
E = 256
BS, NA = 2, 900
ACORE = 225                      # anchors per core
ACH = [(0, 128), (128, 97)]      # anchor chunks within a core
SHAPES = [(64, 176), (32, 88), (16, 44), (8, 22)]
LVL_ROWS = [(h + 1) * (w + 1) for (h, w) in SHAPES]      # 11505,2937,765,207
LBASE = [0, 11505, 14442, 15207]
CAM_ROWS = sum(LVL_ROWS)         # 15414
NR = C * CAM_ROWS                # 92484 rows of 1024
NR4 = NR * 4                     # 369936 quarter-rows of 256
NSLOT = C * L * P                # 312
NSQ = NSLOT * 4                  # 1248
SQCH = [(t * 128, min(128, NSQ - t * 128)) for t in range((NSQ + 127) // 128)]
A_SUB = 6                        # anchors in flight (3 per mega psum tile)
BIG = 8.0e6                      # oob index penalty (stays < 2^24 in fp32)


# packed-constant column layout
PACK_COLS = {}
_c = 0
for _nm, _w in [("ift0", ACORE), ("ift1", ACORE), ("aet0", ACORE), ("aet1", ACORE),
                ("wwfc0", 2496), ("wwfc1", 2496), ("wout0", E), ("wout1", E),
                ("wkps0", 52), ("wkps1", 52), ("iff0", E), ("iff1", E),
                ("mask8", E), ("anc0", ACORE), ("anc1", ACORE), ("anc2", ACORE),
                ("bwfc", 2496), ("bout", E), ("projt", 24), ("bkps", 4),
                ("base6", 6)]:
    PACK_COLS[_nm] = (_c, _w)
    _c += _w
NCOL = _c

_NC = None


def build_nc():
    nc = bass.Bass()
    quad = nc.declare_dram_parameter("quad", [NR4, E], F32, isOutput=False)
    pack_d = nc.declare_dram_parameter("pack", [128, NCOL], F32, isOutput=False)
    out_d = nc.declare_dram_parameter("out", [ACORE, E], F32, isOutput=True)
    kscr_d = nc.dram_tensor("kscratch", [4, 13, ACORE], F32)

    with TileContext(nc) as tc, ExitStack() as ctx:
        cst = ctx.enter_context(tc.tile_pool(name="cst", bufs=1))
        wk = ctx.enter_context(tc.tile_pool(name="wk", bufs=2))
        wk1 = ctx.enter_context(tc.tile_pool(name="wk1", bufs=1))
        gp = ctx.enter_context(tc.tile_pool(name="gp", bufs=3))
        ps1 = ctx.enter_context(tc.tile_pool(name="ps1", bufs=1, space="PSUM"))
        pst = ctx.enter_context(tc.tile_pool(name="pst", bufs=1, space="PSUM"))
        psm = ctx.enter_context(tc.tile_pool(name="psm", bufs=3, space="PSUM"))
        psf = ctx.enter_context(tc.tile_pool(name="psf", bufs=2, space="PSUM"))

        # ---- single packed constant load ----
        pk_t = cst.tile([128, NCOL], F32, tag="pack")
        nc.sync.dma_start(out=pk_t[:], in_=pack_d[:])
        pk = pk_t[:]

        def pcol(name, rows=128):
            c0, cn = PACK_COLS[name]
            return pk[0:rows, c0:c0 + cn]

        ift = [pcol("ift0"), pcol("ift1")]
        aet = [pcol("aet0"), pcol("aet1")]
        iff = [pcol("iff0")[0:ACH[0][1], :], pcol("iff1")[0:ACH[1][1], :]]
        anc = [pcol(f"anc{j}", rows=1) for j in range(3)]
        wkps = [pcol("wkps0"), pcol("wkps1")]
        bkps = pcol("bkps", rows=13)
        projt = pcol("projt", rows=4)
        wwfc = [pcol("wwfc0"), pcol("wwfc1")]
        bwfc = pcol("bwfc", rows=1)
        wout = [pcol("wout0"), pcol("wout1")]
        bout = pcol("bout", rows=1)
        mask8 = pcol("mask8")
        base6 = pcol("base6")

        ident = cst.tile([128, 128], F32, tag="ident")
        make_identity(nc, ident[:])
        ones1 = cst.tile([1, 128], F32, tag="ones1")
        nc.vector.memset(ones1[:], 1.0)
        ones128 = cst.tile([128, 1], F32, tag="ones128")
        nc.vector.memset(ones128[:], 1.0)
        bounds_reg = nc.gpsimd.to_reg(NR4 - 1)

        # ---- keypoints kpJ [4, 13*225] (j on partitions, (p,a) free) ----
        kpj = cst.tile([4, 13 * ACORE], F32, tag="kpj")
        for j in range(4):
            pj = ps1.tile([13, 512], F32, tag="psmisc", space="PSUM")
            nc.tensor.matmul(out=pj[:, :ACORE], lhsT=wkps[0][:, j * 13:(j + 1) * 13],
                             rhs=ift[0], start=True, stop=False)
            nc.tensor.matmul(out=pj[:, :ACORE], lhsT=wkps[1][:, j * 13:(j + 1) * 13],
                             rhs=ift[1], start=False, stop=(j == 3))
            if j < 3:
                nc.tensor.matmul(out=pj[:, :ACORE], lhsT=ones1[:, :13], rhs=anc[j],
                                 start=False, stop=True)
            stg = wk.tile([13, ACORE], F32, tag="kstg")
            nc.scalar.activation(out=stg[:], in_=pj[:, :ACORE], func=ACTF.Identity,
                                 bias=bkps[:, j:j + 1], scale=1.0)
            nc.sync.dma_start(out=kscr_d[j], in_=stg[:])
        nc.sync.dma_start(
            out=kpj[:].rearrange("j (p a) -> j p a", p=13),
            in_=kscr_d[:])

        fusedT = [cst.tile([128, ACORE], F32, tag=f"fusedT{h}", name=f"fusedT{h}") for h in range(2)]

        # ================= per anchor-chunk =================
        for ci, (a0, asz) in enumerate(ACH):
            # ---- projection: p2d [asz, 312] (p,i,c) ----
            pp = ps1.tile([128, 512], F32, tag="psmisc", space="PSUM")
            for p in range(13):
                nc.tensor.matmul(out=pp[:asz, p * 24:(p + 1) * 24],
                                 lhsT=kpj[:, p * ACORE + a0: p * ACORE + a0 + asz],
                                 rhs=projt, start=True, stop=True)
            p2d = wk.tile([128, 312], F32, tag="p2d")
            nc.scalar.copy(out=p2d[:asz, :], in_=pp[:asz, :312])
            pv = p2d[:asz, :].rearrange("a (p i c) -> a p i c", p=13, i=4)

            # ---- wfc matmul + softmax -> wsoft [asz, 2496] ----
            wraw = wk1.tile([128, 2496], F32, tag="wraw")
            nb = [(0, 512), (512, 512), (1024, 512), (1536, 512), (2048, 448)]
            for (n0, nsz) in nb:
                pw_ = ps1.tile([128, 512], F32, tag="pswfc", space="PSUM")
                nc.tensor.matmul(out=pw_[:asz, :nsz], lhsT=ift[0][:, a0:a0 + asz],
                                 rhs=wwfc[0][:, n0:n0 + nsz], start=True, stop=False)
                nc.tensor.matmul(out=pw_[:asz, :nsz], lhsT=ift[1][:, a0:a0 + asz],
                                 rhs=wwfc[1][:, n0:n0 + nsz], start=False, stop=False)
                nc.tensor.matmul(out=pw_[:asz, :nsz], lhsT=aet[0][:, a0:a0 + asz],
                                 rhs=wwfc[0][:, n0:n0 + nsz], start=False, stop=False)
                nc.tensor.matmul(out=pw_[:asz, :nsz], lhsT=aet[1][:, a0:a0 + asz],
                                 rhs=wwfc[1][:, n0:n0 + nsz], start=False, stop=False)
                nc.tensor.matmul(out=pw_[:asz, :nsz], lhsT=ones1[:, :asz],
                                 rhs=bwfc[:, n0:n0 + nsz], start=False, stop=True)
                nc.scalar.copy(out=wraw[:asz, n0:n0 + nsz], in_=pw_[:asz, :nsz])
            wgs = wraw[:asz, :].rearrange("a (s g) -> a g s", g=8)
            wmax = wk.tile([128, 8], F32, tag="wmax")
            nc.vector.tensor_reduce(out=wmax[:asz, :], in_=wgs,
                                    axis=mybir.AxisListType.X, op=OP.max)
            wmb = wmax[:asz, :].unsqueeze(2).broadcast_to([asz, 8, 312])
            nc.vector.tensor_tensor(out=wgs, in0=wgs, in1=wmb, op=OP.subtract)
            nc.scalar.activation(out=wraw[:asz, :], in_=wraw[:asz, :], func=ACTF.Exp)
            wsum = wk.tile([128, 8], F32, tag="wsum")
            nc.vector.tensor_reduce(out=wsum[:asz, :], in_=wgs,
                                    axis=mybir.AxisListType.X, op=OP.add)
            nc.vector.reciprocal(out=wsum[:asz, :], in_=wsum[:asz, :])
            wsb = wsum[:asz, :].unsqueeze(2).broadcast_to([asz, 8, 312])
            nc.vector.tensor_tensor(out=wgs, in0=wgs, in1=wsb, op=OP.mult)
            wsoft = wraw  # now normalized in place

            # ---- per-level pixel math ----
            def coord(tag):
                t = wk1.tile([128, 78], F32, tag=tag, name=tag)
                return t, t[:asz, :].rearrange("a (p c) -> a p c", p=13)

            def pslice(i):
                return pv[:, :, i:i + 1, :].squeeze(2)   # [asz, 13, 6]

            zc_t, zc = coord("zc")
            gx_t, gx = coord("gx")
            gy_t, gy = coord("gy")
            nc.vector.tensor_scalar_max(out=zc, in0=pslice(2), scalar1=1e-5)
            nc.vector.reciprocal(out=zc, in_=zc)
            nc.vector.tensor_tensor(out=gx, in0=pslice(0), in1=zc, op=OP.mult)
            nc.vector.tensor_tensor(out=gy, in0=pslice(1), in1=zc, op=OP.mult)

            pwt = wk1.tile([128, NSQ], F32, tag="pwt")        # (c,l,p,q)
            idx4 = wk1.tile([128, NSQ], F32, tag="idx4")      # (c,l,p,q) fp32
            pwv = pwt[:asz, :].rearrange("a (c l p q) -> a c l p q", c=6, l=4, q=4)
            idv = idx4[:asz, :].rearrange("a (c l p q) -> a c l p q", c=6, l=4, q=4)
            basev = base6[0:asz, :].unsqueeze(1).broadcast_to([asz, 13, 6])

            for l, (H, W) in enumerate(SHAPES):
                ix_t, ix = coord("ix")
                iy_t, iy = coord("iy")
                fx_t, fx = coord("fx")
                fy_t, fy = coord("fy")
                x0_t, x0 = coord("x0")
                y0_t, y0 = coord("y0")
                va_t, va = coord("va")
                vb_t, vb = coord("vb")
                wy0_t, wy0 = coord("wy0")
                wy1_t, wy1 = coord("wy1")
                t1_t, t1 = coord("t1")
                # pixel coords, shifted +16 so trunc == floor; clamp to [6, W+26]
                nc.vector.tensor_scalar(out=ix, in0=gx, scalar1=W / 2.0,
                                        scalar2=(W - 1) / 2.0 + 16.0, op0=OP.mult, op1=OP.add)
                nc.vector.tensor_scalar(out=iy, in0=gy, scalar1=H / 2.0,
                                        scalar2=(H - 1) / 2.0 + 16.0, op0=OP.mult, op1=OP.add)
                nc.vector.tensor_scalar(out=ix, in0=ix, scalar1=float(W + 26),
                                        scalar2=6.0, op0=OP.min, op1=OP.max)
                nc.vector.tensor_scalar(out=iy, in0=iy, scalar1=float(H + 26),
                                        scalar2=6.0, op0=OP.min, op1=OP.max)
                xi = wk1.tile([128, 78], I32, tag="xi", name="xi")
                yi = wk1.tile([128, 78], I32, tag="yi", name="yi")
                nc.vector.tensor_copy(out=xi[:asz, :], in_=ix_t[:asz, :])
                nc.vector.tensor_copy(out=yi[:asz, :], in_=iy_t[:asz, :])
                nc.vector.tensor_copy(out=x0_t[:asz, :], in_=xi[:asz, :])
                nc.vector.tensor_copy(out=y0_t[:asz, :], in_=yi[:asz, :])
                nc.vector.tensor_tensor(out=fx, in0=ix, in1=x0, op=OP.subtract)
                nc.vector.tensor_tensor(out=fy, in0=iy, in1=y0, op=OP.subtract)
                # validity in shifted coords: x0+16 in [15, W+15], y0+16 in [15, H+15]
                nc.vector.tensor_scalar(out=va, in0=x0, scalar1=15.0, scalar2=None, op0=OP.is_ge)
                nc.vector.tensor_scalar(out=vb, in0=x0, scalar1=float(W + 15), scalar2=None, op0=OP.is_le)
                nc.vector.tensor_tensor(out=va, in0=va, in1=vb, op=OP.mult)
                nc.vector.tensor_scalar(out=vb, in0=y0, scalar1=15.0, scalar2=None, op0=OP.is_ge)
                nc.vector.tensor_tensor(out=va, in0=va, in1=vb, op=OP.mult)
                nc.vector.tensor_scalar(out=vb, in0=y0, scalar1=float(H + 15), scalar2=None, op0=OP.is_le)
                nc.vector.tensor_tensor(out=va, in0=va, in1=vb, op=OP.mult)
                # bilinear weights, validity folded into wy
                nc.vector.tensor_scalar(out=wy0, in0=fy, scalar1=-1.0, scalar2=1.0,
                                        op0=OP.mult, op1=OP.add)
                nc.vector.tensor_tensor(out=wy0, in0=wy0, in1=va, op=OP.mult)
                nc.vector.tensor_tensor(out=wy1, in0=fy, in1=va, op=OP.mult)
                # wx0 = 1-fx stored into fy's tile (fy no longer needed)
                wx0 = fy
                nc.vector.tensor_scalar(out=wx0, in0=fx, scalar1=-1.0, scalar2=1.0,
                                        op0=OP.mult, op1=OP.add)
                for q, (wyv, wxv) in enumerate([(wy0, wx0), (wy0, fx), (wy1, wx0), (wy1, fx)]):
                    dst = pwv[:, :, l:l + 1, :, q:q + 1].squeeze(4).squeeze(2).transpose([0, 2, 1])
                    nc.vector.tensor_tensor(out=dst, in0=wyv, in1=wxv, op=OP.mult)
                # index quarter-rows
                nc.vector.scalar_tensor_tensor(out=t1, in0=y0, scalar=float(W + 1),
                                               in1=x0, op0=OP.mult, op1=OP.add)
                nc.vector.tensor_tensor(out=t1, in0=t1, in1=basev, op=OP.add)
                for q in range(4):
                    cq = 4.0 * LBASE[l] + 4 * (W + 1) + 4 + q + 0.5 + BIG - 64.0 * (W + 2)
                    u_t, u = coord("u")
                    nc.vector.tensor_scalar(out=u, in0=t1, scalar1=4.0, scalar2=cq,
                                            op0=OP.mult, op1=OP.add)
                    dst = idv[:, :, l:l + 1, :, q:q + 1].squeeze(4).squeeze(2).transpose([0, 2, 1])
                    nc.vector.scalar_tensor_tensor(out=dst, in0=va, scalar=-BIG,
                                                   in1=u, op0=OP.mult, op1=OP.add)

            # ---- W_final [asz, 9984] = wsoft x pw ----
            wfin = wk1.tile([128, NSLOT * 32], F32, tag="wfin")
            wsv = wsoft[:asz, :].rearrange("a (s g) -> a s g", g=8) \
                .unsqueeze(2).broadcast_to([asz, NSLOT, 4, 8])
            pwv2 = pwt[:asz, :].rearrange("a (s q) -> a s q", q=4) \
                .unsqueeze(3).broadcast_to([asz, NSLOT, 4, 8])
            nc.vector.tensor_tensor(
                out=wfin[:asz, :].rearrange("a (s q g) -> a s q g", q=4, g=8),
                in0=wsv, in1=pwv2, op=OP.mult)

            # ---- transposes: idxT4 (int32) and WtT per sq-chunk ----
            idxT = []
            for t, (s0, ssz) in enumerate(SQCH):
                pt = pst.tile([128, 512], F32, tag="pstr", space="PSUM")
                nc.tensor.transpose(out=pt[:ssz, :asz], in_=idx4[:asz, s0:s0 + ssz],
                                    identity=ident[:asz, :asz])
                it = wk1.tile([128, 128], I32, tag=f"idxT{t}")
                nc.vector.tensor_copy(out=it[:ssz, :asz], in_=pt[:ssz, :asz])
                idxT.append(it)
            wfv = wfin[:asz, :].rearrange("a (sq g) -> a sq g", g=8)
            wtt = []
            for t, (s0, ssz) in enumerate(SQCH):
                wt_t = wk1.tile([128, 128 * 8], F32, tag=f"wtt{t}")
                wtt.append(wt_t)
                for g in range(8):
                    pt = pst.tile([128, 512], F32, tag="pstr", space="PSUM")
                    nc.tensor.transpose(
                        out=pt[:ssz, :asz],
                        in_=wfv[:, s0:s0 + ssz, g:g + 1].squeeze(2),
                        identity=ident[:asz, :asz])
                    nc.vector.tensor_copy(
                        out=wt_t[:ssz, :].rearrange("s (a g) -> s a g", g=8)[:, :asz, g:g + 1].squeeze(2),
                        in_=pt[:ssz, :asz])

            # ---- gather + fusion reduce ----
            psft = [psf.tile([128, 512], F32, tag="psft", name="psft", space="PSUM") for _ in range(2)]
            if ci == 0:
                for _ in range(3):   # zero-init the gather slots once
                    gz = gp.tile([128, A_SUB * E], F32, tag="G")
                    nc.vector.memset(gz[:], 0.0)
            nsub = (asz + A_SUB - 1) // A_SUB
            for si in range(nsub):
                sa0 = si * A_SUB
                ssz_a = min(A_SUB, asz - sa0)
                gts = []
                for t, (s0, ssz) in enumerate(SQCH):
                    gt = gp.tile([128, A_SUB * E], F32, tag="G")
                    gts.append(gt)
                    nc.gpsimd.indirect_dma_start(
                        out=gt[:ssz, :ssz_a * E].rearrange("s (a c) -> s a c", c=E),
                        out_offset=None,
                        in_=quad[:, :],
                        in_offset=bass.IndirectOffsetOnAxis(
                            ap=idxT[t][:ssz, sa0:sa0 + ssz_a], axis=0),
                        bounds_check=bounds_reg,
                        oob_is_err=False)
                megas = [psm.tile([128, 512], F32, tag="mega", name="mega", space="PSUM")
                         for _ in range((ssz_a + 2) // 3)]
                for t, (s0, ssz) in enumerate(SQCH):
                    for al in range(ssz_a):
                        mg = megas[al // 3]
                        mb = (al % 3) * 32
                        nc.tensor.matmul(
                            out=mg[mb:mb + 8, :E],
                            lhsT=wtt[t][:ssz, (sa0 + al) * 8:(sa0 + al) * 8 + 8],
                            rhs=gts[t][:ssz, al * E:(al + 1) * E],
                            start=(t == 0), stop=(t == len(SQCH) - 1),
                            skip_group_check=True)
                for al in range(ssz_a):
                    mg = megas[al // 3]
                    mb = (al % 3) * 32
                    sbf = wk.tile([128, E], F32, tag="sbf")
                    nc.vector.tensor_tensor(out=sbf[mb:mb + 8, :], in0=mg[mb:mb + 8, :E],
                                            in1=mask8[mb:mb + 8, :], op=OP.mult)
                    col = sa0 + al
                    for h in range(2):
                        nc.tensor.matmul(
                            out=psft[h][:, col:col + 1],
                            lhsT=sbf[mb:mb + 8, h * 128:(h + 1) * 128],
                            rhs=ones128[mb:mb + 8, :], start=True, stop=True)
            for h in range(2):
                nc.scalar.copy(out=fusedT[h][:, a0:a0 + asz], in_=psft[h][:, :asz])

        # ---- output projection + residual ----
        for ci, (a0, asz) in enumerate(ACH):
            po = ps1.tile([128, 512], F32, tag="psmisc", space="PSUM")
            nc.tensor.matmul(out=po[:asz, :E], lhsT=fusedT[0][:, a0:a0 + asz],
                             rhs=wout[0], start=True, stop=False)
            nc.tensor.matmul(out=po[:asz, :E], lhsT=fusedT[1][:, a0:a0 + asz],
                             rhs=wout[1], start=False, stop=False)
            nc.tensor.matmul(out=po[:asz, :E], lhsT=ones1[:, :asz],
                             rhs=bout, start=False, stop=True)
            osb = wk.tile([128, E], F32, tag="osb")
            nc.vector.tensor_tensor(out=osb[:asz, :], in0=po[:asz, :E],
                                    in1=iff[ci], op=OP.add)
            nc.sync.dma_start(out=out_d[a0:a0 + asz, :], in_=osb[:asz, :])

    return nc



def _split_excess_waits(nc, max_waits=1):
    """Walrus encodes at most ~2 sync waits per compute instruction; hoist
    excess waits onto chained NoOps on the same engine."""
    fn = nc.m.functions[0]
    ctr = 0
    for blk in fn.blocks:
        new = []
        changed = False
        for inst in blk.instructions:
            si = inst.sync_info
            w = list(si.on_wait) if si is not None and si.on_wait is not None else []
            opname = type(inst).__name__
            if len(w) > max_waits and "EventSemaphore" not in opname:
                excess = w[:len(w) - max_waits]
                for k in range(0, len(excess), max_waits):
                    ctr += 1
                    nop = mybir.InstNoOp(name=f"I-wsplit-{ctr}", ins=[], outs=[])
                    nop.engine = inst.engine
                    nop.sync_info = mybir.SyncInfo(on_wait=excess[k:k + max_waits],
                                                   on_update=[])
                    new.append(nop)
                inst.sync_info = mybir.SyncInfo(
                    on_wait=w[len(w) - max_waits:],
                    on_update=list(si.on_update) if si.on_update is not None else [])
                changed = True
            new.append(inst)
        if changed:
            blk.instructions = new
    return ctr


def get_nc():
    global _NC
    if _NC is None:
        _NC = build_nc()
        _split_excess_waits(_NC)
    return _NC


def build_quad_tables(fm0, fm1, fm2, fm3):
    """[2, NR4, 256] fp32: zero-padded 2x2 corner table per (cam, level)."""
    big = np.zeros((2, NR, 1024), dtype=np.float32)
    for l, fm in enumerate((fm0, fm1, fm2, fm3)):
        H, W = SHAPES[l]
        fmT = np.ascontiguousarray(fm.transpose(0, 2, 3, 1))     # (12,H,W,256)
        P2 = np.zeros((12, H + 2, W + 2, E), dtype=np.float32)
        P2[:, 1:H + 1, 1:W + 1] = fmT
        for b in range(2):
            for c in range(6):
                src = P2[b * 6 + c]
                r0 = c * CAM_ROWS + LBASE[l]
                tgt = big[b, r0:r0 + LVL_ROWS[l]].reshape(H + 1, W + 1, 4, E)
                tgt[:, :, 0] = src[:H + 1, :W + 1]
                tgt[:, :, 1] = src[:H + 1, 1:]
                tgt[:, :, 2] = src[1:, :W + 1]
                tgt[:, :, 3] = src[1:, 1:]
    return big.reshape(2, NR4, E)


MASK8 = np.zeros((128, E), dtype=np.float32)
for _rep in range(4):
    for _g in range(8):
        MASK8[_rep * 32 + _g, _g * 32:(_g + 1) * 32] = 1.0


def build_in_maps(inputs):
    IF = np.asarray(inputs["instance_feature"], dtype=np.float32)
    ANC = np.asarray(inputs["anchor"], dtype=np.float32)
    AE = np.asarray(inputs["anchor_embed"], dtype=np.float32)
    proj = np.asarray(inputs["projection_mat"], dtype=np.float32)
    W_kps = np.asarray(inputs["W_kps"], dtype=np.float32)
    b_kps = np.asarray(inputs["b_kps"], dtype=np.float32)
    quad = build_quad_tables(*[np.asarray(inputs[f"fm{i}"], dtype=np.float32)
                               for i in range(4)])
    # W_kps: (256, 39) cols (p,j) -> (j,p)-major padded to 52
    Wr = np.zeros((E, 4, 13), dtype=np.float32)
    Wr[:, :3, :] = W_kps.reshape(E, 13, 3).transpose(0, 2, 1)
    Wr = Wr.reshape(E, 52)
    bk = np.zeros((13, 4), dtype=np.float32)
    bk[:, :3] = b_kps.reshape(13, 3)
    bk[:, 3] = 1.0
    in_maps = []
    for core in range(8):
        b, blk = core // 4, core % 4
        sl = slice(blk * ACORE, (blk + 1) * ACORE)
        pT = np.ascontiguousarray(proj[b].transpose(2, 1, 0).reshape(4, 24))
        pack = np.zeros((128, NCOL), dtype=np.float32)

        def put(name, arr):
            c0, cn = PACK_COLS[name]
            a = np.asarray(arr, dtype=np.float32)
            pack[0:a.shape[0], c0:c0 + a.shape[1]] = a

        IFT = IF[b, sl].T          # (256, 225)
        AET = AE[b, sl].T
        put("ift0", IFT[0:128]); put("ift1", IFT[128:256])
        put("aet0", AET[0:128]); put("aet1", AET[128:256])
        W_wfc = np.asarray(inputs["W_wfc"], dtype=np.float32)
        put("wwfc0", W_wfc[0:128]); put("wwfc1", W_wfc[128:256])
        W_out = np.asarray(inputs["W_out"], dtype=np.float32)
        put("wout0", W_out[0:128]); put("wout1", W_out[128:256])
        put("wkps0", Wr[0:128]); put("wkps1", Wr[128:256])
        put("iff0", IF[b, sl][0:128]); put("iff1", IF[b, sl][128:225])
        put("mask8", MASK8)
        for j in range(3):
            put(f"anc{j}", ANC[b, sl, j:j + 1].T)
        put("bwfc", np.asarray(inputs["b_wfc"], dtype=np.float32)[None, :])
        put("bout", np.asarray(inputs["b_out"], dtype=np.float32)[None, :])
        put("projt", pT)
        put("bkps", bk)
        put("base6", np.tile(np.arange(6, dtype=np.float32)[None, :] * CAM_ROWS,
                             (128, 1)))
        in_maps.append({"quad": quad[b], "pack": pack})
    return in_maps


def run_cores(inputs, **kw):
    nc = get_nc()
    in_maps = build_in_maps(inputs)
    return run_bass_kernel_spmd(nc, in_maps, list(range(8)), **kw)


def kernel(**inputs):
    br = run_cores(inputs)
    out = np.empty((BS, NA, E), dtype=np.float32)
    for core in range(8):
        b, blk = core // 4, core % 4
        out[b, blk * ACORE:(blk + 1) * ACORE] = br.results[core]["out"]
    return out



# revision 2
# speedup vs baseline: 2.0825x; 2.0825x over previous
"""Deformable feature aggregation kernel v2 for 8 Trainium2 NeuronCores.

Strategy (self-contained, hardcoded for the fixed problem shapes):
  - Shard (bs=2 x NA=900) -> 8 cores x 225 anchors; core c handles batch
    c//4, anchor block c%4. Feature maps: each core gets its batch's half.
  - Host prep: feature maps repacked into a bf16 zero-padded half-quad
    table [2, C*HROWS, 512]: row (2*r+h) holds the two x-corner pixel
    vectors of quad row r's y-corner h (1KB contiguous).
  - Device: keypoint/projection/softmax matmuls on PE; per-level pixel math
    on DVE; gather via gpsimd dma_gather with int16 per-cam indices (sample
    j = a*128 + lp*2 + h so each 128-partition chunk is one anchor); fusion
    via per-(anchor,cam,qx) matmuls lhsT=[128,8] bf16 weights x rhs=[128,256]
    gathered bf16 features accumulating [8,256] per anchor in PSUM; masked
    extract + ones-matmul builds fusedT; output projection; residual.
"""

import numpy as np
from contextlib import ExitStack

import ml_dtypes
import concourse.bass as bass
import concourse.mybir as mybir
import concourse.tile as tile
from concourse.tile import TileContext
from concourse.masks import make_identity
from concourse.bass_utils import run_bass_kernel_spmd
from concourse import library_config

F32 = mybir.dt.float32
BF16 = mybir.dt.bfloat16
I16 = mybir.dt.int16
I32 = mybir.dt.int32
OP = mybir.AluOpType
ACTF = mybir.ActivationFunctionType

# problem constants
C, L, P, G = 6, 4, 13, 8
E = 256
BS, NA = 2, 900
ACORE = 225                      # anchors per core
ACH = [(0, 128), (128, 97)]      # anchor chunks within a core
SHAPES = [(64, 176), (32, 88), (16, 44), (8, 22)]
LVL_ROWS = [(h + 1) * (w + 1) for (h, w) in SHAPES]      # 11505,2937,765,207
LBASE = [0, 11505, 14442, 15207]
CAM_ROWS = sum(LVL_ROWS)         # 15414 quad rows per cam
NR = C * CAM_ROWS                # 92484 quad rows per batch
HROWS = CAM_ROWS * 2             # 30828 half rows per cam (int16-safe)
LP = 52                          # slots per (anchor, cam) = 4 levels * 13 pts
V = 128                          # padded (lp, h) space per (anchor, cam)
GA = 16                          # anchors per gather group
NGRP = (ACORE + GA - 1) // GA    # 15 groups (last has 1)
IDXW = ACORE * 8                 # idx cols per cam


# packed-constant column layout
PACK_COLS = {}
_c = 0
for _nm, _w in [("ift0", ACORE), ("ift1", ACORE), ("aet0", ACORE), ("aet1", ACORE),
                ("wwfc0", 2496), ("wwfc1", 2496), ("wout0", E), ("wout1", E),
                ("wkps0", 52), ("wkps1", 52), ("iff0", E), ("iff1", E),
                ("mask512", 512), ("anc0", ACORE), ("anc1", ACORE), ("anc2", ACORE),
                ("bwfc", 2496), ("bout", E), ("projt", 24), ("bkps", 4)]:
    PACK_COLS[_nm] = (_c, _w)
    _c += _w
NCOL = _c

_NC = None

import os
STOP = int(os.environ.get("KV2_STOP", "9"))   # debug: stop after stage N


def build_nc():
    nc = bass.Bass()
    quad = nc.declare_dram_parameter("quad", [C * HROWS, 512], BF16, isOutput=False)
    pack_d = nc.declare_dram_parameter("pack", [128, NCOL], F32, isOutput=False)
    out_d = nc.declare_dram_parameter("out", [ACORE, E], F32, isOutput=True)
    kscr_d = nc.dram_tensor("kscratch", [4, 13, ACORE], F32)

    with TileContext(nc) as tc, ExitStack() as ctx:
        cst = ctx.enter_context(tc.tile_pool(name="cst", bufs=1))
        wk = ctx.enter_context(tc.tile_pool(name="wk", bufs=2))
        wk1 = ctx.enter_context(tc.tile_pool(name="wk1", bufs=1))
        gp = ctx.enter_context(tc.tile_pool(name="gp", bufs=2))
        wtp = ctx.enter_context(tc.tile_pool(name="wtp", bufs=1))
        exp_ = ctx.enter_context(tc.tile_pool(name="exp", bufs=2))
        ps1 = ctx.enter_context(tc.tile_pool(name="ps1", bufs=1, space="PSUM"))
        psacc = ctx.enter_context(tc.tile_pool(name="psacc", bufs=3, space="PSUM"))
        psft_p = ctx.enter_context(tc.tile_pool(name="psftp", bufs=1, space="PSUM"))

        nc.gpsimd.load_library(library_config.mlp)

        # ---- single packed constant load ----
        pk_t = cst.tile([128, NCOL], F32, tag="pack")
        nc.sync.dma_start(out=pk_t[:], in_=pack_d[:])
        pk = pk_t[:]

        def pcol(name, rows=128):
            c0, cn = PACK_COLS[name]
            return pk[0:rows, c0:c0 + cn]

        ift = [pcol("ift0"), pcol("ift1")]
        aet = [pcol("aet0"), pcol("aet1")]
        iff = [pcol("iff0")[0:ACH[0][1], :], pcol("iff1")[0:ACH[1][1], :]]
        anc = [pcol(f"anc{j}", rows=1) for j in range(3)]
        wkps = [pcol("wkps0"), pcol("wkps1")]
        bkps = pcol("bkps", rows=13)
        projt = pcol("projt", rows=4)
        wwfc = [pcol("wwfc0"), pcol("wwfc1")]
        bwfc = pcol("bwfc", rows=1)
        wout = [pcol("wout0"), pcol("wout1")]
        bout = pcol("bout", rows=1)
        mask512 = pcol("mask512")

        ident = cst.tile([128, 128], F32, tag="ident")
        make_identity(nc, ident[:])
        ones1 = cst.tile([1, 128], F32, tag="ones1")
        nc.vector.memset(ones1[:], 1.0)
        ones128 = cst.tile([128, 1], F32, tag="ones128")
        nc.vector.memset(ones128[:], 1.0)

        # ---- keypoints kpJ [4, 13*225] (j on partitions, (p,a) free) ----
        kpj = cst.tile([4, 13 * ACORE], F32, tag="kpj")
        for j in range(4):
            pj = ps1.tile([128, 512], F32, tag="psmisc", space="PSUM")
            nc.tensor.matmul(out=pj[:13, :ACORE], lhsT=wkps[0][:, j * 13:(j + 1) * 13],
                             rhs=ift[0], start=True, stop=False)
            nc.tensor.matmul(out=pj[:13, :ACORE], lhsT=wkps[1][:, j * 13:(j + 1) * 13],
                             rhs=ift[1], start=False, stop=(j == 3))
            if j < 3:
                nc.tensor.matmul(out=pj[:13, :ACORE], lhsT=ones1[:, :13], rhs=anc[j],
                                 start=False, stop=True)
            stg = wk.tile([13, ACORE], F32, tag="kstg")
            nc.scalar.activation(out=stg[:], in_=pj[:13, :ACORE], func=ACTF.Identity,
                                 bias=bkps[:, j:j + 1], scale=1.0)
            nc.sync.dma_start(out=kscr_d[j], in_=stg[:])
        nc.sync.dma_start(
            out=kpj[:].rearrange("j (p a) -> j p a", p=13),
            in_=kscr_d[:])

        # persistent per-chunk tensors
        wsoft = [cst.tile([128, 2496], F32, tag=f"wsoft{h}", name=f"wsoft{h}")
                 for h in range(2)]
        pwt_t = [cst.tile([128, C * LP * 4], F32, tag=f"pwt{h}", name=f"pwt{h}")
                 for h in range(2)]
        idxA = [cst.tile([128, C * V], F32, tag=f"idxA{h}", name=f"idxA{h}")
                for h in range(2)]
        fusedT = [cst.tile([128, ACORE], F32, tag=f"fusedT{h}", name=f"fusedT{h}")
                  for h in range(2)]
        idx16 = cst.tile([128, C * IDXW], I16, tag="idx16", name="idx16")

        # ================= per anchor-chunk: weights + indices =================
        for ci, (a0, asz) in enumerate(ACH):
            # ---- projection: p2d [asz, 312] (p,i,c) ----
            pp = ps1.tile([128, 512], F32, tag="psmisc", space="PSUM")
            for p in range(13):
                nc.tensor.matmul(out=pp[:asz, p * 24:(p + 1) * 24],
                                 lhsT=kpj[:, p * ACORE + a0: p * ACORE + a0 + asz],
                                 rhs=projt, start=True, stop=True)
            p2d = wk.tile([128, 312], F32, tag="p2d")
            nc.scalar.copy(out=p2d[:asz, :], in_=pp[:asz, :312])
            pv = p2d[:asz, :].rearrange("a (p i c) -> a p i c", p=13, i=4)

            # ---- wfc matmul + softmax -> wsoft [asz, 2496] ----
            wraw = wsoft[ci]
            nb = [(0, 512), (512, 512), (1024, 512), (1536, 512), (2048, 448)]
            for (n0, nsz) in nb:
                pw_ = ps1.tile([128, 512], F32, tag="pswfc", space="PSUM")
                nc.tensor.matmul(out=pw_[:asz, :nsz], lhsT=ift[0][:, a0:a0 + asz],
                                 rhs=wwfc[0][:, n0:n0 + nsz], start=True, stop=False)
                nc.tensor.matmul(out=pw_[:asz, :nsz], lhsT=ift[1][:, a0:a0 + asz],
                                 rhs=wwfc[1][:, n0:n0 + nsz], start=False, stop=False)
                nc.tensor.matmul(out=pw_[:asz, :nsz], lhsT=aet[0][:, a0:a0 + asz],
                                 rhs=wwfc[0][:, n0:n0 + nsz], start=False, stop=False)
                nc.tensor.matmul(out=pw_[:asz, :nsz], lhsT=aet[1][:, a0:a0 + asz],
                                 rhs=wwfc[1][:, n0:n0 + nsz], start=False, stop=False)
                nc.tensor.matmul(out=pw_[:asz, :nsz], lhsT=ones1[:, :asz],
                                 rhs=bwfc[:, n0:n0 + nsz], start=False, stop=True)
                nc.scalar.copy(out=wraw[:asz, n0:n0 + nsz], in_=pw_[:asz, :nsz])
            wgs = wraw[:asz, :].rearrange("a (s g) -> a g s", g=8)
            wmax = wk.tile([128, 8], F32, tag="wmax")
            nc.vector.tensor_reduce(out=wmax[:asz, :], in_=wgs,
                                    axis=mybir.AxisListType.X, op=OP.max)
            wmb = wmax[:asz, :].unsqueeze(2).broadcast_to([asz, 8, 312])
            nc.vector.tensor_tensor(out=wgs, in0=wgs, in1=wmb, op=OP.subtract)
            nc.scalar.activation(out=wraw[:asz, :], in_=wraw[:asz, :], func=ACTF.Exp)
            wsum = wk.tile([128, 8], F32, tag="wsum")
            nc.vector.tensor_reduce(out=wsum[:asz, :], in_=wgs,
                                    axis=mybir.AxisListType.X, op=OP.add)
            nc.vector.reciprocal(out=wsum[:asz, :], in_=wsum[:asz, :])
            wsb = wsum[:asz, :].unsqueeze(2).broadcast_to([asz, 8, 312])
            nc.vector.tensor_tensor(out=wgs, in0=wgs, in1=wsb, op=OP.mult)
            # wsoft[ci] now holds normalized weights, cols ((c l p) g)

            # ---- per-level pixel math -> pwt (bilinear w) + idxA (half-rows) ----
            nc.vector.memset(idxA[ci][:], 0.0)
            pwv = pwt_t[ci][:asz, :].rearrange("a (c l p q) -> a c l p q", c=6, l=4, q=4)
            idv = idxA[ci][:asz, :].rearrange("a (c lp h) -> a c lp h", c=6, h=2)

            def coord(tag):
                t = wk1.tile([128, 78], F32, tag=tag, name=tag)
                return t, t[:asz, :].rearrange("a (p c) -> a p c", p=13)

            def pslice(i):
                return pv[:, :, i:i + 1, :].squeeze(2)   # [asz, 13, 6]

            zc_t, zc = coord("zc")
            gx_t, gx = coord("gx")
            gy_t, gy = coord("gy")
            nc.vector.tensor_scalar_max(out=zc, in0=pslice(2), scalar1=1e-5)
            nc.vector.reciprocal(out=zc, in_=zc)
            nc.vector.tensor_tensor(out=gx, in0=pslice(0), in1=zc, op=OP.mult)
            nc.vector.tensor_tensor(out=gy, in0=pslice(1), in1=zc, op=OP.mult)

            for l, (H, W) in enumerate(SHAPES):
                ix_t, ix = coord("ix")
                iy_t, iy = coord("iy")
                fx_t, fx = coord("fx")
                fy_t, fy = coord("fy")
                x0_t, x0 = coord("x0")
                y0_t, y0 = coord("y0")
                va_t, va = coord("va")
                vb_t, vb = coord("vb")
                wy0_t, wy0 = coord("wy0")
                wy1_t, wy1 = coord("wy1")
                rw_t, rw = coord("rw")
                # pixel coords shifted +16 so trunc == floor
                nc.vector.tensor_scalar(out=ix, in0=gx, scalar1=W / 2.0,
                                        scalar2=(W - 1) / 2.0 + 16.0, op0=OP.mult, op1=OP.add)
                nc.vector.tensor_scalar(out=iy, in0=gy, scalar1=H / 2.0,
                                        scalar2=(H - 1) / 2.0 + 16.0, op0=OP.mult, op1=OP.add)
                # validity from unclamped coords: x0=floor(ix)-16 in [-1,W-1]
                nc.vector.tensor_scalar(out=va, in0=ix, scalar1=15.0, scalar2=None, op0=OP.is_ge)
                nc.vector.tensor_scalar(out=vb, in0=ix, scalar1=W + 15.9999, scalar2=None, op0=OP.is_le)
                nc.vector.tensor_tensor(out=va, in0=va, in1=vb, op=OP.mult)
                nc.vector.tensor_scalar(out=vb, in0=iy, scalar1=15.0, scalar2=None, op0=OP.is_ge)
                nc.vector.tensor_tensor(out=va, in0=va, in1=vb, op=OP.mult)
                nc.vector.tensor_scalar(out=vb, in0=iy, scalar1=H + 15.9999, scalar2=None, op0=OP.is_le)
                nc.vector.tensor_tensor(out=va, in0=va, in1=vb, op=OP.mult)
                # clamp so floor lands in [15, W+15] -> table col in [0, W]
                nc.vector.tensor_scalar(out=ix, in0=ix, scalar1=float(W + 15),
                                        scalar2=15.0, op0=OP.min, op1=OP.max)
                nc.vector.tensor_scalar(out=iy, in0=iy, scalar1=float(H + 15),
                                        scalar2=15.0, op0=OP.min, op1=OP.max)
                xi = wk1.tile([128, 78], I32, tag="xi", name="xi")
                yi = wk1.tile([128, 78], I32, tag="yi", name="yi")
                nc.vector.tensor_copy(out=xi[:asz, :], in_=ix_t[:asz, :])
                nc.vector.tensor_copy(out=yi[:asz, :], in_=iy_t[:asz, :])
                nc.vector.tensor_copy(out=x0_t[:asz, :], in_=xi[:asz, :])
                nc.vector.tensor_copy(out=y0_t[:asz, :], in_=yi[:asz, :])
                nc.vector.tensor_tensor(out=fx, in0=ix, in1=x0, op=OP.subtract)
                nc.vector.tensor_tensor(out=fy, in0=iy, in1=y0, op=OP.subtract)
                # bilinear weights, validity folded into wy
                nc.vector.tensor_scalar(out=wy0, in0=fy, scalar1=-1.0, scalar2=1.0,
                                        op0=OP.mult, op1=OP.add)
                nc.vector.tensor_tensor(out=wy0, in0=wy0, in1=va, op=OP.mult)
                nc.vector.tensor_tensor(out=wy1, in0=fy, in1=va, op=OP.mult)
                wx0 = fy  # reuse tile
                nc.vector.tensor_scalar(out=wx0, in0=fx, scalar1=-1.0, scalar2=1.0,
                                        op0=OP.mult, op1=OP.add)
                for q, (wyv, wxv) in enumerate([(wy0, wx0), (wy0, fx), (wy1, wx0), (wy1, fx)]):
                    dst = pwv[:, :, l:l + 1, :, q:q + 1].squeeze(4).squeeze(2).transpose([0, 2, 1])
                    nc.vector.tensor_tensor(out=dst, in0=wyv, in1=wxv, op=OP.mult)
                # half-row index: rw = y0*(W+1) + x0 (shifted); idx2 = 2*row + h
                nc.vector.scalar_tensor_tensor(out=rw, in0=y0, scalar=float(W + 1),
                                               in1=x0, op0=OP.mult, op1=OP.add)
                cst_off = 2.0 * (LBASE[l] - 15.0 * (W + 1) - 15.0)
                for h in range(2):
                    dst = idv[:, :, l * 13:(l + 1) * 13, h:h + 1].squeeze(3) \
                        .transpose([0, 2, 1])
                    nc.vector.tensor_scalar(out=dst, in0=rw, scalar1=2.0,
                                            scalar2=cst_off + h, op0=OP.mult, op1=OP.add)

        # ================= idx16 transposes + replication =================
        for cam in range(C if STOP >= 2 else 0):
            for ci, (a0, asz) in enumerate(ACH):
                dv = idx16[0:16, cam * IDXW + a0 * 8: cam * IDXW + (a0 + asz) * 8] \
                    .rearrange("p (a v) -> p v a", v=8)
                for vh in range(2):
                    pt = ps1.tile([128, 512], F32, tag="pstr", space="PSUM", bufs=2)
                    for gi in range(4):
                        vg = vh * 4 + gi
                        nc.tensor.transpose(
                            out=pt[0:16, gi * 128: gi * 128 + asz],
                            in_=idxA[ci][:asz, cam * V + vg * 16: cam * V + (vg + 1) * 16],
                            identity=ident[:asz, :asz])
                    for gi in range(4):
                        vg = vh * 4 + gi
                        nc.vector.tensor_copy(
                            out=dv[:, vg:vg + 1, :].squeeze(1),
                            in_=pt[0:16, gi * 128: gi * 128 + asz])
        for k in range(1, 8 if STOP >= 3 else 1):
            nc.sync.dma_start(out=idx16[k * 16:(k + 1) * 16, :], in_=idx16[0:16, :])

        # ================= per chunk-phase: wT build + gather + fusion =========
        if STOP >= 7:
            psft = psft_p.tile([128, 512], F32, tag="psftt", name="psft",
                               space="PSUM")
        else:
            psft = None
        nreg = {}
        for grp in range(NGRP):
            gasz = min(GA, ACORE - grp * GA)
            for sub in range(0, gasz, 8):
                n = min(8, gasz - sub) * 128
                if n not in nreg:
                    r = nc.gpsimd.to_reg(n)
                    nreg[n] = r

        for ci, (a0c, aszc) in enumerate(ACH if STOP >= 4 else []):
            # ---- transposed fusion weights for this chunk ----
            # 24 zero pad cols so a [128, 32] lhsT slice is valid for the
            # last anchors (extra rows land in masked-out PSUM rows).
            wTc = [[wtp.tile([128, aszc * 8 + 24], BF16, tag=f"wT{c}_{qx}",
                             name=f"wT{c}_{qx}_{ci}")
                    for qx in range(2)] for c in range(C)]
            for cam in range(C):
                for qx in range(2):
                    nc.vector.memset(wTc[cam][qx][:, aszc * 8:], 0.0)
                    asz = aszc
                    w8 = wk.tile([128, 1024], F32, tag="w8")
                    nc.vector.memset(w8[:, LP * 16:], 0.0)
                    wsv = wsoft[ci][:asz, cam * 416:(cam + 1) * 416] \
                        .rearrange("a (lp g) -> a lp g", g=8) \
                        .unsqueeze(2).broadcast_to([asz, LP, 2, 8])
                    pwv2 = pwt_t[ci][:asz, cam * 208:(cam + 1) * 208] \
                        .rearrange("a (lp h x) -> a lp h x", h=2, x=2)[:, :, :, qx:qx + 1] \
                        .squeeze(3).unsqueeze(3).broadcast_to([asz, LP, 2, 8])
                    nc.vector.tensor_tensor(
                        out=w8[:asz, :LP * 16].rearrange("a (lp h g) -> a lp h g", h=2, g=8),
                        in0=wsv, in1=pwv2, op=OP.mult)
                    w8g = w8[:asz, :].rearrange("a (v g) -> a g v", g=8)
                    for gh in range(2):
                        pt = ps1.tile([128, 512], F32, tag="pstr", space="PSUM", bufs=2)
                        for gi in range(4):
                            g = gh * 4 + gi
                            nc.tensor.transpose(
                                out=pt[:, gi * 128: gi * 128 + asz],
                                in_=w8g[:, g:g + 1, :].squeeze(1),
                                identity=ident[:asz, :asz])
                        nc.vector.tensor_copy(
                            out=wTc[cam][qx][:, :asz * 8]
                                .rearrange("p (a g) -> p a g", g=8)[:, :, gh * 4:(gh + 1) * 4],
                            in_=pt[:, :512].rearrange("p (gi a) -> p a gi", a=128)[:, :asz, :])

            # ---- groups of this chunk ----
            g_lo = a0c // GA
            g_hi = (a0c + aszc + GA - 1) // GA if STOP >= 5 else a0c // GA
            for grp in range(g_lo, g_hi):
                ga0 = grp * GA
                gasz = min(GA, ACORE - ga0)
                banks = [psacc.tile([128, 512], F32, tag="acc", name="acc", space="PSUM")
                         for _ in range((gasz + 7) // 8)]
                if gasz < GA:
                    for bk in banks:
                        nc.vector.memset(bk[:], 0.0)
                gts = []
                for cam in range(C):
                    gt = gp.tile([128, GA * 512], BF16, tag="G")
                    gts.append(gt)
                    # <=1024 idx per call (SWDGE ring limit)
                    for sub in range(0, gasz, 8):
                        nsub = min(8, gasz - sub)
                        a0i = (ga0 + sub) * 8
                        nc.gpsimd.dma_gather(
                            gt[:, sub * 512:(sub + nsub) * 512]
                                .rearrange("p (a e) -> p a e", e=512),
                            quad[cam * HROWS:(cam + 1) * HROWS, :],
                            idx16[:, cam * IDXW + a0i: cam * IDXW + a0i + nsub * 8],
                            nsub * 128,
                            nreg[nsub * 128],
                            512,
                            elem_step=512)
                for cam in range(C if STOP >= 6 else 0):
                    gt = gts[cam]
                    for al in range(gasz):
                        a = ga0 + al
                        loc = a - a0c
                        bk = banks[al // 8]
                        base = (al % 4) * 32
                        half = (al // 4) % 2
                        for qx in range(2):
                            nc.tensor.matmul(
                                out=bk[base:base + 32, half * 256:(half + 1) * 256],
                                lhsT=wTc[cam][qx][:, loc * 8:loc * 8 + 32],
                                rhs=gt[:, al * 512 + qx * 256: al * 512 + (qx + 1) * 256],
                                start=(cam == 0 and qx == 0),
                                stop=(cam == C - 1 and qx == 1),
                                tile_position=(0, base),
                                skip_group_check=True)
                # ---- extract group -> fusedT columns ----
                for bi, bk in enumerate(banks if STOP >= 7 else []):
                    exs = exp_.tile([128, 512], F32, tag="exs")
                    nc.vector.tensor_tensor(out=exs[:], in0=bk[:], in1=mask512,
                                            op=OP.mult)
                    for al in range(bi * 8, min(gasz, bi * 8 + 8)):
                        a = ga0 + al
                        base = (al % 4) * 32
                        half = (al // 4) % 2
                        for h in range(2):
                            nc.tensor.matmul(
                                out=psft[:, h * 256 + a: h * 256 + a + 1],
                                lhsT=exs[base:base + 8,
                                         half * 256 + h * 128: half * 256 + (h + 1) * 128],
                                rhs=ones128[base:base + 8, :],
                                start=True, stop=True,
                                tile_position=(base, 0),
                                skip_group_check=True)

        for h in range(2):
            if STOP >= 7:
                nc.scalar.copy(out=fusedT[h][:, :],
                               in_=psft[:, h * 256: h * 256 + ACORE])
            else:
                nc.vector.memset(fusedT[h][:], 0.0)

        # ---- output projection + residual ----
        for ci, (a0, asz) in enumerate(ACH):
            po = ps1.tile([128, 512], F32, tag="psmisc", space="PSUM")
            nc.tensor.matmul(out=po[:asz, :E], lhsT=fusedT[0][:, a0:a0 + asz],
                             rhs=wout[0], start=True, stop=False)
            nc.tensor.matmul(out=po[:asz, :E], lhsT=fusedT[1][:, a0:a0 + asz],
                             rhs=wout[1], start=False, stop=False)
            nc.tensor.matmul(out=po[:asz, :E], lhsT=ones1[:, :asz],
                             rhs=bout, start=False, stop=True)
            osb = wk.tile([128, E], F32, tag="osb")
            nc.vector.tensor_tensor(out=osb[:asz, :], in0=po[:asz, :E],
                                    in1=iff[ci], op=OP.add)
            nc.sync.dma_start(out=out_d[a0:a0 + asz, :], in_=osb[:asz, :])

    return nc


def _split_excess_waits(nc, max_waits=1):
    """Walrus encodes at most ~2 sync waits per compute instruction; hoist
    excess waits onto chained NoOps on the same engine."""
    fn = nc.m.functions[0]
    ctr = 0
    for blk in fn.blocks:
        new = []
        changed = False
        for inst in blk.instructions:
            si = inst.sync_info
            w = list(si.on_wait) if si is not None and si.on_wait is not None else []
            opname = type(inst).__name__
            if len(w) > max_waits and "EventSemaphore" not in opname:
                excess = w[:len(w) - max_waits]
                for k in range(0, len(excess), max_waits):
                    ctr += 1
                    nop = mybir.InstNoOp(name=f"I-wsplit-{ctr}", ins=[], outs=[])
                    nop.engine = inst.engine
                    nop.sync_info = mybir.SyncInfo(on_wait=excess[k:k + max_waits],
                                                   on_update=[])
                    new.append(nop)
                inst.sync_info = mybir.SyncInfo(
                    on_wait=w[len(w) - max_waits:],
                    on_update=list(si.on_update) if si.on_update is not None else [])
                changed = True
            new.append(inst)
        if changed:
            blk.instructions = new
    return ctr


def get_nc():
    global _NC
    if _NC is None:
        _NC = build_nc()
        _split_excess_waits(_NC)
        # encode InstPseudoReloadLibraryIndex (library load for dma_gather)
        mybir.codegen_inst_isa_subclasses(_NC)
    return _NC


def _to_bf16(a):
    """fp32 -> bf16 uint16 bits with round-to-nearest-even."""
    u = a.view(np.uint32)
    return ((u + 0x7FFF + ((u >> 16) & 1)) >> 16).astype(np.uint16)


def build_quad_tables(fm0, fm1, fm2, fm3):
    """[2, C*HROWS, 512] bf16: per (cam,level) half-quad rows."""
    big = np.zeros((2, NR, 1024), dtype=np.uint16)
    for l, fm in enumerate((fm0, fm1, fm2, fm3)):
        H, W = SHAPES[l]
        fmT = np.ascontiguousarray(fm.transpose(0, 2, 3, 1))     # (12,H,W,256)
        P2 = np.zeros((12, H + 2, W + 2, E), dtype=np.uint16)
        P2[:, 1:H + 1, 1:W + 1] = _to_bf16(fmT)
        for b in range(2):
            for c in range(6):
                src = P2[b * 6 + c]
                r0 = c * CAM_ROWS + LBASE[l]
                tgt = big[b, r0:r0 + LVL_ROWS[l]].reshape(H + 1, W + 1, 4, E)
                tgt[:, :, 0] = src[:H + 1, :W + 1]
                tgt[:, :, 1] = src[:H + 1, 1:]
                tgt[:, :, 2] = src[1:, :W + 1]
                tgt[:, :, 3] = src[1:, 1:]
    return big.reshape(2, C * HROWS, 512).view(ml_dtypes.bfloat16)


def _make_mask512():
    m = np.zeros((128, 512), dtype=np.float32)
    for b in range(4):
        for g in range(8):
            for half in range(2):
                m[b * 32 + g, half * 256 + g * 32:half * 256 + (g + 1) * 32] = 1.0
    return m


def build_in_maps(inputs):
    IF = np.asarray(inputs["instance_feature"], dtype=np.float32)
    ANC = np.asarray(inputs["anchor"], dtype=np.float32)
    AE = np.asarray(inputs["anchor_embed"], dtype=np.float32)
    proj = np.asarray(inputs["projection_mat"], dtype=np.float32)
    W_kps = np.asarray(inputs["W_kps"], dtype=np.float32)
    b_kps = np.asarray(inputs["b_kps"], dtype=np.float32)
    quad = build_quad_tables(*[np.asarray(inputs[f"fm{i}"], dtype=np.float32)
                               for i in range(4)])
    # W_kps: (256, 39) cols (p,j) -> (j,p)-major padded to 52
    Wr = np.zeros((E, 4, 13), dtype=np.float32)
    Wr[:, :3, :] = W_kps.reshape(E, 13, 3).transpose(0, 2, 1)
    Wr = Wr.reshape(E, 52)
    bk = np.zeros((13, 4), dtype=np.float32)
    bk[:, :3] = b_kps.reshape(13, 3)
    bk[:, 3] = 1.0
    mask512 = _make_mask512()
    in_maps = []
    for core in range(8):
        b, blk = core // 4, core % 4
        sl = slice(blk * ACORE, (blk + 1) * ACORE)
        pT = np.ascontiguousarray(proj[b].transpose(2, 1, 0).reshape(4, 24))
        pack = np.zeros((128, NCOL), dtype=np.float32)

        def put(name, arr):
            c0, cn = PACK_COLS[name]
            a = np.asarray(arr, dtype=np.float32)
            pack[0:a.shape[0], c0:c0 + a.shape[1]] = a

        IFT = IF[b, sl].T          # (256, 225)
        AET = AE[b, sl].T
        put("ift0", IFT[0:128]); put("ift1", IFT[128:256])
        put("aet0", AET[0:128]); put("aet1", AET[128:256])
        W_wfc = np.asarray(inputs["W_wfc"], dtype=np.float32)
        put("wwfc0", W_wfc[0:128]); put("wwfc1", W_wfc[128:256])
        W_out = np.asarray(inputs["W_out"], dtype=np.float32)
        put("wout0", W_out[0:128]); put("wout1", W_out[128:256])
        put("wkps0", Wr[0:128]); put("wkps1", Wr[128:256])
        put("iff0", IF[b, sl][0:128]); put("iff1", IF[b, sl][128:225])
        put("mask512", mask512)
        for j in range(3):
            put(f"anc{j}", ANC[b, sl, j:j + 1].T)
        put("bwfc", np.asarray(inputs["b_wfc"], dtype=np.float32)[None, :])
        put("bout", np.asarray(inputs["b_out"], dtype=np.float32)[None, :])
        put("projt", pT)
        put("bkps", bk)
        in_maps.append({"quad": quad[b], "pack": pack})
    return in_maps


def run_cores(inputs, **kw):
    nc = get_nc()
    in_maps = build_in_maps(inputs)
    return run_bass_kernel_spmd(nc, in_maps, list(range(8)), **kw)


def kernel(**inputs):
    br = run_cores(inputs)
    out = np.empty((BS, NA, E), dtype=np.float32)
    for core in range(8):
        b, blk = core // 4, core % 4
        out[b, blk * ACORE:(blk + 1) * ACORE] = br.results[core]["out"]
    return out
